# revision 6
# baseline (speedup 1.0000x reference)
"""Trainium2 Bass kernel for nn_MultiHeadCrossAttention (B=16, Dq=768, H=12,
hd=64, Nq=1024, Nt=64, Dkv=384) with RoPE on q and k.

Sharding: pure data-parallel over batch, 2 batches per core across 8 cores.
No collectives.

v5 design (v4 + DMA-descriptor/trigger economics):
  - Every dma_start costs its issuing engine ~600-900ns, and small strided
    transfers are descriptor-rate-bound (~170ns per ~1KB descriptor per
    queue). So: all small constants ride ONE packed byte-blob DMA (bitcast
    views on SBUF), tokT is a direct HBM load again (the v4 SBUF->SBUF
    assembly cost ~5us in tiny descriptors), kz zeros come from DRAM
    instead of a 2.7us gpsimd memset, the wq x featb interleave is 2
    triggers, and the output is ONE contiguous 786KB write per chunk
    (v3/v4 wrote [128,2,512] slices: 256 1KB-descriptors each, ~33us of
    aggregate queue time that drained as a ~13us kernel tail).
  - sigma (the k-side rotate_half partition swap) runs on a fused
    [128, 2, NPAIR, BL, NT] tile: 4 DMAs total instead of 8.
  - k-RoPE t1/t2 multiplies run on DVE (Pool took 1.6us each), and the
    kz scatter is emitted mid-qproj so DVE finishes it right when the
    qk0 matmuls need kz.
  - Residual+bias ride the PE: featres (= feat + bias, bf16) accumulates
    into the out-projection PSUM group via an identity bf16 matmul.
  - q/out projections: fp8 DoubleRow over contraction-tile pairs;
    scores: one zero-padded block-diagonal DR matmul per head pair;
    attn@V: zero-padded block-diagonal single fp8 matmuls;
    softmax denominator: DR reduction with a 16-col padded 0/1 lhsT,
    reciprocal broadcast via small PE matmuls.
"""

import os
import sys
from contextlib import ExitStack

import numpy as np

sys.path.insert(0, "/opt/trn_rl_repo")

import concourse.bass as bass  # noqa: E402
import concourse.mybir as mybir  # noqa: E402
import concourse.tile as tile  # noqa: E402
from concourse import bacc  # noqa: E402
from concourse.bass_utils import run_bass_kernel_spmd  # noqa: E402

import ml_dtypes

F32 = mybir.dt.float32
BF16 = mybir.dt.bfloat16
F8 = mybir.dt.float8e4
U8 = mybir.dt.uint8
NPBF = ml_dtypes.bfloat16
NPF8 = ml_dtypes.float8_e4m3

B, DQ, T, HP, WP = 16, 768, 4, 16, 16
NQ = T * HP * WP            # 1024
NT, DKV = 64, 384
H, HD = 12, 64
SCALE = HD ** -0.5
NCORES = 8
BL = B // NCORES            # batches per core = 2
CHUNK = 512                 # query positions per chunk
NCH = NQ // CHUNK           # chunks per batch = 2
KQ = DQ // 128              # 6 contraction tiles for Dq
KKV = DKV // 128            # 3 contraction tiles for Dkv
NPAIR = H // 2              # 6 head pairs
DR = mybir.MatmulPerfMode.DoubleRow

# packed constant blob: per-partition byte offsets
_O_EPS, _O_NEPS = 0, 4
_O_CK, _O_SK = 8, 136
_O_DLHS = 264
_O_IDENT = 360
_O_BLHS = 616
_O_CTAB = 2152
_O_STAB = 4200
_NBLOB = 6248


def _rope_tables(n):
    inv_freq = 1.0 / (10000.0 ** (np.arange(0, HD, 2, dtype=np.float64) / HD))
    freqs = np.arange(n, dtype=np.float64)[:, None] * inv_freq[None, :]
    emb = np.concatenate([freqs, freqs], axis=-1)  # [n, 64]
    return (np.cos(emb).T.astype(np.float32), np.sin(emb).T.astype(np.float32))


def _const_blob():
    cq, sq = _rope_tables(NQ)          # [64, 1024]
    ck, sk = _rope_tables(NT)          # [64, 64]
    # attention scale split sqrt/sqrt between the q and k tables so both fp8
    # score operands sit in the normal (non-denormal) fp8e4 range
    ss = SCALE ** 0.5
    ctab = (np.concatenate([cq, cq], axis=0) * ss).astype(NPBF)   # [128, NQ]
    stab = (np.concatenate([sq, sq], axis=0) * ss).astype(NPBF)
    ck2 = (np.concatenate([ck, ck], axis=0) * ss).astype(NPBF)    # [128, NT]
    sk2 = (np.concatenate([sk, sk], axis=0) * ss).astype(NPBF)
    eps64 = np.where(np.arange(HD) < HD // 2, -1.0, 1.0).astype(np.float32)
    eps2 = np.concatenate([eps64, eps64])[:, None]                # [128, 1]
    # denominator lhsT: for pair j, col 2j sums partitions 0-63 (even head),
    # col 2j+1 sums partitions 64-127 (odd head). Padded to 16 output
    # columns (dual-fp8 ldweights reject M=12); pad cols get a single 1 so
    # their reciprocal stays finite.
    dlhs = np.zeros((128, NPAIR, 16), np.float32)
    for j in range(NPAIR):
        dlhs[:64, j, 2 * j] = 1.0
        dlhs[64:, j, 2 * j + 1] = 1.0
    dlhs[0, 0, H:] = 1.0
    # broadcast lhsT: for pair j, row 2j feeds cols 0-63, row 2j+1 cols 64-127
    blhs = np.zeros((128, NPAIR, 128), np.float32)
    for j in range(NPAIR):
        blhs[2 * j, j, :64] = 1.0
        blhs[2 * j + 1, j, 64:] = 1.0
    ident = np.eye(128, dtype=np.float32)

    blob = np.zeros((128, _NBLOB), np.uint8)

    def put(off, arr):
        b = np.ascontiguousarray(arr).view(np.uint8).reshape(128, -1)
        blob[:, off:off + b.shape[1]] = b

    put(_O_EPS, eps2.astype(np.float32))
    put(_O_NEPS, (-eps2).astype(np.float32))
    put(_O_CK, ck2)
    put(_O_SK, sk2)
    put(_O_DLHS, dlhs.astype(NPF8))
    put(_O_IDENT, ident.astype(NPBF))
    put(_O_BLHS, blhs.astype(NPBF))
    put(_O_CTAB, ctab)
    put(_O_STAB, stab)
    return blob


def _sigma_dma(nc, out_ap, in_ap, eng):
    """out = sigma(in): swap the 32-partition halves inside each 64-block
    (the RoPE rotate_half permutation). SBUF->SBUF DMA on the given ring."""
    for dst, src in ((0, 32), (32, 0), (64, 96), (96, 64)):
        eng.dma_start(out=out_ap[dst:dst + 32], in_=in_ap[src:src + 32])


def build(debug=False):
    nc = bacc.Bacc("TRN2" if debug else None,
                   target_bir_lowering=False, debug=debug)
    with tile.TileContext(nc) as tc:
        with tc.tile_pool(name="dram", bufs=1, space="DRAM") as dram:
            def din(name, shape, dt=F32):
                return dram.tile(shape, dt, kind="ExternalInput", name=name,
                                 uniquify=False)

            featq = din("featq", [BL, NCH, 128, KQ, CHUNK], F8)
            featres = din("featres", [BL, NCH, 128, KQ, CHUNK], BF16)
            tokT_l = din("tokT_l", [128, KKV, BL, NT], F8)
            tokTz_l = din("tokTz_l", [128, KKV, BL, 2, 128], F8)
            wq = din("wq", [128, KQ, DQ], F8)
            wk = din("wk", [128, KKV, DQ], F8)
            wvz = din("wvz", [128, KKV, 2, NPAIR // 2, 256], F8)
            wout = din("wout", [128, KQ, DQ], F8)
            blob_l = din("blob", [128, _NBLOB], U8)
            kz0_l = din("kz0", [128, NPAIR, 2, BL, 128], F8)
            out_l = dram.tile([BL, NCH, 128, KQ, CHUNK], BF16,
                              kind="ExternalOutput", name="out_l",
                              uniquify=False)

            with ExitStack() as body_ctx:
                global _body_ctx
                _body_ctx = body_ctx
                _body(nc, tc, featq, featres, tokT_l, tokTz_l, wq, wk, wvz,
                      wout, blob_l, kz0_l, out_l)
    nc.compile()
    return nc


def _body(nc, tc, featq, featres, tokT_l, tokTz_l, wq, wk, wvz, wout,
          blob_l, kz0_l, out_l):
    MULT = mybir.AluOpType.mult
    ADD = mybir.AluOpType.add
    EXP = mybir.ActivationFunctionType.Exp

    ctx = _body_ctx
    consts = ctx.enter_context(tc.tile_pool(name="consts", bufs=1))
    kside = ctx.enter_context(tc.tile_pool(name="kside", bufs=1))
    featp = ctx.enter_context(tc.tile_pool(name="featp", bufs=2))
    frp = ctx.enter_context(tc.tile_pool(name="frp", bufs=2))
    qp = ctx.enter_context(tc.tile_pool(name="qp", bufs=2))
    qsbp = ctx.enter_context(tc.tile_pool(name="qsbp", bufs=3))
    ep = ctx.enter_context(tc.tile_pool(name="ep", bufs=2))
    atp = ctx.enter_context(tc.tile_pool(name="atp", bufs=2))
    outp = ctx.enter_context(tc.tile_pool(name="outp", bufs=2))
    rp = ctx.enter_context(tc.tile_pool(name="rp", bufs=2))

    # single-bank tiles with a 4-deep rotation decouple the PE from the
    # DVE/ACT consumer latency
    pp2 = ctx.enter_context(tc.tile_pool(name="pp2", bufs=4, space="PSUM"))
    attn = ctx.enter_context(tc.tile_pool(name="attn", bufs=3, space="PSUM"))
    dp = ctx.enter_context(tc.tile_pool(name="dp", bufs=1, space="PSUM"))

    # ---- constant loads ------------------------------------------------
    # sync ring: the PE-critical path in need-order; gpsimd ring: blob +
    # kz zeros + per-chunk featres + output writes; scalar ring: compute
    # only (ktd/vbd copies, EXP, staging).
    tokT_sb = consts.tile([128, KKV, BL, NT], F8)
    nc.sync.dma_start(out=tokT_sb, in_=tokT_l[:])
    wk_sb = consts.tile([128, KKV, DQ], F8)
    nc.sync.dma_start(out=wk_sb, in_=wk[:])

    blob_sb = consts.tile([128, _NBLOB], U8)
    nc.gpsimd.dma_start(out=blob_sb, in_=blob_l[:])
    eps_sb = blob_sb[:, _O_EPS:_O_EPS + 4].bitcast(F32)
    neps_sb = blob_sb[:, _O_NEPS:_O_NEPS + 4].bitcast(F32)
    ck2_sb = blob_sb[:, _O_CK:_O_CK + 2 * NT].bitcast(BF16)
    sk2_sb = blob_sb[:, _O_SK:_O_SK + 2 * NT].bitcast(BF16)
    dlhs_sb = blob_sb[:, _O_DLHS:_O_DLHS + NPAIR * 16].bitcast(F8) \
        .rearrange("p (j m) -> p j m", j=NPAIR)
    ident_sb = blob_sb[:, _O_IDENT:_O_IDENT + 256].bitcast(BF16)
    blhs_sb = blob_sb[0:H, _O_BLHS:_O_BLHS + NPAIR * 256].bitcast(BF16) \
        .rearrange("p (j m) -> p j m", j=NPAIR)
    ctab_sb = blob_sb[:, _O_CTAB:_O_CTAB + 2 * NQ].bitcast(BF16)
    stab_sb = blob_sb[:, _O_STAB:_O_STAB + 2 * NQ].bitcast(BF16)

    wq_sb = consts.tile([128, KQ, DQ], F8)
    wout_sb = consts.tile([128, KQ, DQ], F8)
    tokTz_sb = consts.tile([128, KKV, BL, 2, 128], F8)
    wvz_sb = consts.tile([128, KKV, 2, NPAIR // 2, 256], F8)

    chunks = [(b, c) for b in range(BL) for c in range(NCH)]
    st = {}
    state = {}

    # ---- pipeline stages ------------------------------------------------
    def stage_qproj(i, mid_cb=None):
        b, c = chunks[i]
        p0 = c * CHUNK
        featb = featp.tile([128, KQ, CHUNK], F8, tag="featb", name=f"fb{i}")
        if i == 0:
            nc.sync.dma_start(out=wq_sb, in_=wq[:])
        nc.sync.dma_start(out=featb, in_=featq[b, c])
        # residual source for this chunk's out-projection (gpsimd ring)
        fres = frp.tile([128, KQ, CHUNK], BF16, tag="fres", name=f"fr{i}")
        nc.gpsimd.dma_start(out=fres, in_=featres[b, c])
        qcs = qp.tile([128, NPAIR, 2, CHUNK], F8, tag="qcs", name=f"qcs{i}")
        for m in range(KQ):
            qps = pp2.tile([128, CHUNK], F32, tag="pp2", name=f"qp{i}_{m}")
            for t in range(KQ // 2):
                nc.tensor.matmul(qps,
                                 wq_sb[:, 2 * t:2 * t + 2,
                                       m * 128:(m + 1) * 128],
                                 featb[:, 2 * t:2 * t + 2, :],
                                 start=(t == 0), stop=(t == KQ // 2 - 1),
                                 perf_mode=DR)
            if m < 4:
                nc.vector.tensor_mul(qcs[:, m, 0, :], qps,
                                     ctab_sb[:, p0:p0 + CHUNK])
                nc.vector.tensor_mul(qcs[:, m, 1, :], qps,
                                     stab_sb[:, p0:p0 + CHUNK])
            else:
                qsb = qsbp.tile([128, CHUNK], BF16, tag="qsb",
                                name=f"qsb{i}_{m}")
                nc.scalar.copy(out=qsb, in_=qps)
                nc.vector.tensor_mul(qcs[:, m, 0, :], qsb,
                                     ctab_sb[:, p0:p0 + CHUNK])
                nc.vector.tensor_mul(qcs[:, m, 1, :], qsb,
                                     stab_sb[:, p0:p0 + CHUNK])
            if m == 2 and mid_cb is not None:
                mid_cb()
        st[i] = dict(qcs=qcs, fres=fres)

    def stage_qk(i):
        b, c = chunks[i]
        s = st[i]
        qcs = s["qcs"]
        kz_sb = state["kz"]
        e_sb = ep.tile([128, NPAIR, CHUNK], F8, tag="e", name=f"e{i}")
        dps = dp.tile([16, CHUNK], F32, tag="den", name=f"d{i}")

        def qk1(j):
            # one zero-padded block-diagonal DR matmul per head pair:
            # k-tile 0 = blockdiag(kA_2j, kA_2j+1), k-tile 1 = kB pair;
            # rhs k-tiles are the natural [q*cos], [q*sin] pair blocks
            sps = attn.tile([128, CHUNK], F32, tag="attn", name=f"s{i}_{j}")
            nc.tensor.matmul(sps, kz_sb[:, j, :, b, :], qcs[:, j, :, :],
                             start=True, stop=True, perf_mode=DR)
            nc.scalar.activation(out=e_sb[:, j, :], in_=sps, func=EXP)

        def denom(t):
            nc.tensor.matmul(dps, dlhs_sb[:, 2 * t:2 * t + 2, :],
                             e_sb[:, 2 * t:2 * t + 2, :],
                             start=(t == 0), stop=(t == NPAIR // 2 - 1),
                             perf_mode=DR)

        for j in range(NPAIR):
            qk1(j)
            if j % 2 == 1:
                denom((j - 1) // 2)
        s["e"], s["dps"] = e_sb, dps

    def stage_recip(i):
        s = st[i]
        r32 = rp.tile([16, CHUNK], F32, tag="r32", name=f"r32_{i}")
        nc.vector.reciprocal_approx_fast(out=r32, in_=s["dps"])
        r_sb = rp.tile([16, CHUNK], BF16, tag="r", name=f"r{i}")
        nc.scalar.copy(out=r_sb, in_=r32)
        s["r"] = r_sb

    def stage_avbc(i):
        b, c = chunks[i]
        s = st[i]
        e_sb, r_sb = s["e"], s["r"]
        vbd_sb = state["vbd"]
        attnT = atp.tile([128, NPAIR, CHUNK], F8, tag="attnT", name=f"at{i}")

        def av(j):
            aps = attn.tile([128, CHUNK], F32, tag="attn", name=f"a{i}_{j}")
            nc.tensor.matmul(aps, vbd_sb[:, b, j, :], e_sb[:, j, :],
                             start=True, stop=True)
            return aps

        def bcast(j):
            bps = attn.tile([128, CHUNK], F32, tag="attn", name=f"b{i}_{j}")
            nc.tensor.matmul(bps, blhs_sb[:, j, :], r_sb[0:H, :],
                             start=True, stop=True)
            # stage to SBUF (DVE may read only one PSUM operand)
            bcs = rp.tile([128, CHUNK], F32, tag="bcs", bufs=3,
                          name=f"bc{i}_{j}")
            if j < 4:
                nc.scalar.copy(out=bcs, in_=bps)
            else:
                nc.vector.tensor_copy(bcs, bps)
            return bcs

        av_t, bc_t = {}, {}
        for j in range(NPAIR):
            av_t[j] = av(j)
            bc_t[j] = bcast(j)
            if j >= 1:
                nc.vector.tensor_mul(attnT[:, j - 1, :],
                                     av_t[j - 1], bc_t[j - 1])
        nc.vector.tensor_mul(attnT[:, NPAIR - 1, :],
                             av_t[NPAIR - 1], bc_t[NPAIR - 1])
        s["attnT"] = attnT

    def stage_oproj(i):
        b, c = chunks[i]
        s = st[i]
        attnT = s["attnT"]
        fres = s["fres"]
        o2 = outp.tile([128, KQ, CHUNK], BF16, tag="o2", name=f"o{i}")
        for m in range(KQ):
            ops = pp2.tile([128, CHUNK], F32, tag="pp2", name=f"op{i}_{m}")
            for t in range(KQ // 2):
                nc.tensor.matmul(ops,
                                 wout_sb[:, 2 * t:2 * t + 2,
                                         m * 128:(m + 1) * 128],
                                 attnT[:, 2 * t:2 * t + 2, :],
                                 start=(t == 0), stop=False,
                                 perf_mode=DR)
            # residual + bias ride the PE: accumulate featres via identity
            nc.tensor.matmul(ops, ident_sb, fres[:, m, :],
                             start=False, stop=True)
            nc.scalar.copy(out=o2[:, m, :], in_=ops)
        # one contiguous 786KB write per chunk
        nc.gpsimd.dma_start(out=out_l[b, c], in_=o2)

    # ---- emission -------------------------------------------------------
    def scoped(fn, tag, i):
        with nc.named_scope(f"{tag}{i}"):
            fn(i)

    _ph0 = nc.named_scope("ph0")
    _ph0.__enter__()
    # kz zero prefill from DRAM (cheaper than a 2.7us gpsimd memset)
    kz_sb = kside.tile([128, NPAIR, 2, BL, 128], F8, tag="kz")
    nc.gpsimd.dma_start(out=kz_sb, in_=kz0_l[:])
    state["kz"] = kz_sb
    # kT as fp8 DR pair + single per head-pair block (12 instructions)
    ktd_sb = kside.tile([128, NPAIR, BL, NT], BF16)
    for hp in range(NPAIR):
        kp = pp2.tile([128, 2, 128], F32, tag="pp2", name=f"kp{hp}")
        nc.tensor.matmul(kp[:, 0, :],
                         wk_sb[:, 0:2, hp * 128:(hp + 1) * 128],
                         tokT_sb[:, 0:2],
                         start=True, stop=False, perf_mode=DR)
        nc.tensor.matmul(kp[:, 0, :],
                         wk_sb[:, 2, hp * 128:(hp + 1) * 128],
                         tokT_sb[:, 2], start=False, stop=True)
        nc.scalar.copy(out=ktd_sb[:, hp], in_=kp[:, 0, :])
    _ph0.__exit__(None, None, None)

    # k RoPE t1/t2 on DVE (in front of chunk-0's q-RoPE in the queue); the
    # k tables are read through 0-stride broadcast APs (pair, batch axes)
    _ph1 = nc.named_scope("ph1")
    _ph1.__enter__()
    ck_v = ck2_sb.unsqueeze(1).unsqueeze(2).broadcast_to([128, NPAIR, BL, NT])
    sk_v = sk2_sb.unsqueeze(1).unsqueeze(2).broadcast_to([128, NPAIR, BL, NT])
    t12 = kside.tile([128, 2, NPAIR, BL, NT], BF16, tag="t12")
    nc.vector.tensor_mul(t12[:, 0], ktd_sb, ck_v)
    nc.vector.tensor_mul(t12[:, 1], ktd_sb, sk_v)
    t12s = kside.tile([128, 2, NPAIR, BL, NT], BF16, tag="t12s")
    _ph1.__exit__(None, None, None)

    def kz_scatter(b):
        # kA = t1 + eps*sigma(t2) into k-tile 0; kB = t2 - eps*sigma(t1)
        # into k-tile 1 (TensorScalarPtr is DVE-only)
        for half in range(2):
            pm = slice(64 * half, 64 * half + 64)
            nc.vector.scalar_tensor_tensor(
                out=kz_sb[pm, :, 0, b, pm], in0=t12s[pm, 1, :, b, :],
                scalar=eps_sb[pm], in1=t12[pm, 0, :, b, :],
                op0=MULT, op1=ADD)
            nc.vector.scalar_tensor_tensor(
                out=kz_sb[pm, :, 1, b, pm], in0=t12s[pm, 0, :, b, :],
                scalar=neps_sb[pm], in1=t12[pm, 1, :, b, :],
                op0=MULT, op1=ADD)

    def ph1_mid():
        # sigma DMAs (gpsimd ring) + batch-0 scatter, emitted mid-qproj so
        # DVE finishes kz right when the qk0 matmuls need it
        with nc.named_scope("ph1b"):
            _sigma_dma(nc, t12s, t12, eng=nc.gpsimd)
            kz_scatter(0)

    scoped(lambda i: stage_qproj(i, mid_cb=ph1_mid), "qp", 0)

    # V-side after qp0 so its sync-ring loads trail wq/featb chunk 0
    _ph2 = nc.named_scope("ph2")
    _ph2.__enter__()
    nc.sync.dma_start(out=tokTz_sb, in_=tokTz_l[:])
    nc.sync.dma_start(out=wvz_sb, in_=wvz[:])
    nc.sync.dma_start(out=wout_sb, in_=wout[:])
    # V: zero-padded block-diagonal DR matmuls (tokens duplicated across the
    # two k-tiles host-side), two head pairs per matmul (N=256) ->
    # vbd = [v_even | 0; 0 | v_odd] per pair.
    vbd_sb = kside.tile([128, BL, NPAIR, 128], F8, tag="vbd")
    state["vbd"] = vbd_sb
    for b in range(BL):
        for g in range(NPAIR // 2):
            vbp = pp2.tile([128, 2, 128], F32, tag="pp2", name=f"vb{b}_{g}")
            for kc in range(KKV):
                nc.tensor.matmul(vbp[:], tokTz_sb[:, kc, b, :, :],
                                 wvz_sb[:, kc, :, g, :],
                                 start=(kc == 0), stop=(kc == KKV - 1),
                                 perf_mode=DR)
            nc.scalar.copy(out=vbd_sb[:, b, 2 * g:2 * g + 2, :], in_=vbp[:])
    _ph2.__exit__(None, None, None)

    scoped(stage_qk, "qk", 0)

    n = len(chunks)
    for i in range(n):
        scoped(stage_recip, "rc", i)
        scoped(stage_avbc, "av", i)
        if i + 1 < n:
            scoped(stage_qproj, "qp", i + 1)
            if i == 0:
                with nc.named_scope("kz1"):
                    kz_scatter(1)
            scoped(stage_qk, "qk", i + 1)
        scoped(stage_oproj, "op", i)


_NC_CACHE = {}


def _get_nc():
    if "nc" not in _NC_CACHE:
        _NC_CACHE["nc"] = build(debug=False)
    return _NC_CACHE["nc"]


def _prep_in_maps(feat, tokens, Wq, Wkv, Wout, bout):
    feat = np.ascontiguousarray(feat, dtype=np.float32).reshape(B, DQ, NQ)
    tokens = np.ascontiguousarray(tokens, dtype=np.float32)
    wv = np.asarray(Wkv)[:, DQ:]                       # [DKV, DQ]
    wvz = np.zeros((KKV, 128, NPAIR, 2, 128), np.float32)
    wvr = wv.reshape(KKV, 128, H, HD)
    for j in range(NPAIR):
        wvz[:, :, j, 0, :HD] = wvr[:, :, 2 * j, :]
        wvz[:, :, j, 1, HD:] = wvr[:, :, 2 * j + 1, :]
    # [kc, p, j, t, 128] -> [p, kc, t, g, 256] with two head pairs per group
    wvz2 = wvz.transpose(1, 0, 3, 2, 4).reshape(128, KKV, 2, NPAIR // 2, 256)
    shared = dict(
        wq=np.ascontiguousarray(
            np.asarray(Wq).reshape(KQ, 128, DQ).transpose(1, 0, 2))
            .astype(NPF8),
        wk=np.ascontiguousarray(
            np.asarray(Wkv)[:, :DQ].reshape(KKV, 128, DQ).transpose(1, 0, 2))
            .astype(NPF8),
        wvz=np.ascontiguousarray(wvz2).astype(NPF8),
        wout=np.ascontiguousarray(
            np.asarray(Wout).reshape(KQ, 128, DQ).transpose(1, 0, 2))
            .astype(NPF8),
        blob=_const_blob(),
        kz0=np.zeros((128, NPAIR, 2, BL, 128), NPF8),
    )
    bias = np.asarray(bout, dtype=np.float32).reshape(KQ, 128)  # [kc, p]
    in_maps = []
    for cid in range(NCORES):
        sl = slice(BL * cid, BL * (cid + 1))
        fl = np.ascontiguousarray(
            feat[sl].reshape(BL, KQ, 128, NQ).transpose(0, 2, 1, 3))
        fo = fl + bias.T[None, :, :, None]
        # per-chunk contiguous layout [BL, NCH, 128, KQ, CHUNK]
        fq = fl.reshape(BL, 128, KQ, NCH, CHUNK).transpose(0, 3, 1, 2, 4)
        fo = fo.reshape(BL, 128, KQ, NCH, CHUNK).transpose(0, 3, 1, 2, 4)
        tk = tokens[sl].reshape(BL, NT, KKV, 128).transpose(3, 2, 0, 1)
        tkz = np.zeros((128, KKV, BL, 2, 128), np.float32)
        tkz[:, :, :, 0, :NT] = tk
        tkz[:, :, :, 1, NT:] = tk
        in_maps.append(dict(
            featq=np.ascontiguousarray(fq).astype(NPF8),
            featres=np.ascontiguousarray(fo).astype(NPBF),
            tokT_l=np.ascontiguousarray(tk).astype(NPF8),
            tokTz_l=np.ascontiguousarray(tkz).astype(NPF8),
            **shared))
    return in_maps


def _install_ntff_hook():
    """The container's antenv lacks axon_hooks; register the NTFF profile
    hook from trn_agent_boot ourselves so trace=True yields HW exec times."""
    import types

    import antenv
    from trn_agent_boot.trn_boot import _ntff_profile_via_ctypes

    mod = types.ModuleType("antenv.axon_hooks")
    state = {"hook": None}
    mod.set_axon_ntff_profile_hook = lambda h: state.__setitem__("hook", h)
    mod.get_axon_ntff_profile_hook = lambda: state["hook"]
    sys.modules["antenv.axon_hooks"] = mod
    antenv.axon_hooks = mod
    mod.set_axon_ntff_profile_hook(
        _ntff_profile_via_ctypes("/opt/axon/libaxon_pjrt.so"))
    # the S3 artifact upload has no credentials here; make it a no-op
    import concourse.bass_utils as bu
    bu.upload_artifacts = lambda tmpdir: f"local:{tmpdir}"


def run(inputs, trace=False, trace_cores=None):
    nc = _get_nc()
    if trace:
        try:
            _install_ntff_hook()
        except Exception as e:  # profiling is best-effort
            print(f"ntff hook install failed: {e}", file=sys.stderr)
            trace = False
    in_maps = _prep_in_maps(**inputs)
    res = run_bass_kernel_spmd(nc, in_maps, core_ids=list(range(NCORES)),
                               trace=trace, trace_cores=trace_cores)
    outs = []
    for r in res.results:
        ol = np.asarray(r["out_l"]).astype(np.float32)
        # [BL, NCH, 128, KQ, CHUNK] -> [BL, DQ, NQ]
        ol = ol.transpose(0, 3, 2, 1, 4).reshape(BL, DQ, NQ)
        outs.append(ol.reshape(BL, DQ, T, HP, WP))
    return np.ascontiguousarray(np.concatenate(outs, axis=0)), res


def kernel(**inputs):
    return run(inputs, trace=False)[0]


# revision 10
# speedup vs baseline: 1.0170x; 1.0170x over previous
"""Trainium2 Bass kernel for nn_MultiHeadCrossAttention (B=16, Dq=768, H=12,
hd=64, Nq=1024, Nt=64, Dkv=384) with RoPE on q and k.

Sharding: pure data-parallel over batch, 2 batches per core across 8 cores.
No collectives.

v5 design (v4 + DMA-descriptor/trigger economics):
  - Every dma_start costs its issuing engine ~600-900ns, and small strided
    transfers are descriptor-rate-bound (~170ns per ~1KB descriptor per
    queue). So: all small constants ride ONE packed byte-blob DMA (bitcast
    views on SBUF), tokT is a direct HBM load again (the v4 SBUF->SBUF
    assembly cost ~5us in tiny descriptors), kz zeros come from DRAM
    instead of a 2.7us gpsimd memset, the wq x featb interleave is 2
    triggers, and the output is ONE contiguous 786KB write per chunk
    (v3/v4 wrote [128,2,512] slices: 256 1KB-descriptors each, ~33us of
    aggregate queue time that drained as a ~13us kernel tail).
  - sigma (the k-side rotate_half partition swap) runs on a fused
    [128, 2, NPAIR, BL, NT] tile: 4 DMAs total instead of 8.
  - k-RoPE t1/t2 multiplies run on DVE (Pool took 1.6us each), and the
    kz scatter is emitted mid-qproj so DVE finishes it right when the
    qk0 matmuls need kz.
  - Residual+bias ride the PE: featres (= feat + bias, bf16) accumulates
    into the out-projection PSUM group via an identity bf16 matmul.
  - q/out projections: fp8 DoubleRow over contraction-tile pairs;
    scores: one zero-padded block-diagonal DR matmul per head pair;
    attn@V: zero-padded block-diagonal single fp8 matmuls;
    softmax denominator: DR reduction with a 16-col padded 0/1 lhsT,
    reciprocal broadcast via small PE matmuls.
"""

import os
import sys
from contextlib import ExitStack

import numpy as np

sys.path.insert(0, "/opt/trn_rl_repo")

import concourse.bass as bass  # noqa: E402
import concourse.mybir as mybir  # noqa: E402
import concourse.tile as tile  # noqa: E402
from concourse import bacc  # noqa: E402
from concourse.bass_utils import run_bass_kernel_spmd  # noqa: E402

import ml_dtypes

F32 = mybir.dt.float32
BF16 = mybir.dt.bfloat16
F8 = mybir.dt.float8e4
U8 = mybir.dt.uint8
NPBF = ml_dtypes.bfloat16
NPF8 = ml_dtypes.float8_e4m3

B, DQ, T, HP, WP = 16, 768, 4, 16, 16
NQ = T * HP * WP            # 1024
NT, DKV = 64, 384
H, HD = 12, 64
SCALE = HD ** -0.5
NCORES = 8
BL = B // NCORES            # batches per core = 2
CHUNK = 512                 # query positions per chunk
NCH = NQ // CHUNK           # chunks per batch = 2
KQ = DQ // 128              # 6 contraction tiles for Dq
KKV = DKV // 128            # 3 contraction tiles for Dkv
NPAIR = H // 2              # 6 head pairs
DR = mybir.MatmulPerfMode.DoubleRow

# packed constant blob: per-partition byte offsets
_O_EPS, _O_NEPS = 0, 4
_O_CK, _O_SK = 8, 136
_O_DLHS = 264
_O_IDENT = 360
_O_BLHS = 616
_O_CTAB = 2152
_O_STAB = 4200
_NBLOB = 6248


def _rope_tables(n):
    inv_freq = 1.0 / (10000.0 ** (np.arange(0, HD, 2, dtype=np.float64) / HD))
    freqs = np.arange(n, dtype=np.float64)[:, None] * inv_freq[None, :]
    emb = np.concatenate([freqs, freqs], axis=-1)  # [n, 64]
    return (np.cos(emb).T.astype(np.float32), np.sin(emb).T.astype(np.float32))


def _const_blob():
    cq, sq = _rope_tables(NQ)          # [64, 1024]
    ck, sk = _rope_tables(NT)          # [64, 64]
    # attention scale split sqrt/sqrt between the q and k tables so both fp8
    # score operands sit in the normal (non-denormal) fp8e4 range
    ss = SCALE ** 0.5
    ctab = (np.concatenate([cq, cq], axis=0) * ss).astype(NPBF)   # [128, NQ]
    stab = (np.concatenate([sq, sq], axis=0) * ss).astype(NPBF)
    ck2 = (np.concatenate([ck, ck], axis=0) * ss).astype(NPBF)    # [128, NT]
    sk2 = (np.concatenate([sk, sk], axis=0) * ss).astype(NPBF)
    eps64 = np.where(np.arange(HD) < HD // 2, -1.0, 1.0).astype(np.float32)
    eps2 = np.concatenate([eps64, eps64])[:, None]                # [128, 1]
    # denominator lhsT: for pair j, col 2j sums partitions 0-63 (even head),
    # col 2j+1 sums partitions 64-127 (odd head). Padded to 16 output
    # columns (dual-fp8 ldweights reject M=12); pad cols get a single 1 so
    # their reciprocal stays finite.
    dlhs = np.zeros((128, NPAIR, 16), np.float32)
    for j in range(NPAIR):
        dlhs[:64, j, 2 * j] = 1.0
        dlhs[64:, j, 2 * j + 1] = 1.0
    dlhs[0, 0, H:] = 1.0
    # broadcast lhsT: for pair j, row 2j feeds cols 0-63, row 2j+1 cols 64-127
    blhs = np.zeros((128, NPAIR, 128), np.float32)
    for j in range(NPAIR):
        blhs[2 * j, j, :64] = 1.0
        blhs[2 * j + 1, j, 64:] = 1.0
    ident = np.eye(128, dtype=np.float32)

    blob = np.zeros((128, _NBLOB), np.uint8)

    def put(off, arr):
        b = np.ascontiguousarray(arr).view(np.uint8).reshape(128, -1)
        blob[:, off:off + b.shape[1]] = b

    put(_O_EPS, eps2.astype(np.float32))
    put(_O_NEPS, (-eps2).astype(np.float32))
    put(_O_CK, ck2)
    put(_O_SK, sk2)
    put(_O_DLHS, dlhs.astype(NPF8))
    put(_O_IDENT, ident.astype(NPBF))
    put(_O_BLHS, blhs.astype(NPBF))
    put(_O_CTAB, ctab)
    put(_O_STAB, stab)
    return blob


def _sigma_dma(nc, out_ap, in_ap, eng):
    """out = sigma(in): swap the 32-partition halves inside each 64-block
    (the RoPE rotate_half permutation). SBUF->SBUF DMA on the given ring."""
    for dst, src in ((0, 32), (32, 0), (64, 96), (96, 64)):
        eng.dma_start(out=out_ap[dst:dst + 32], in_=in_ap[src:src + 32])


def build(debug=False):
    nc = bacc.Bacc("TRN2" if debug else None,
                   target_bir_lowering=False, debug=debug)
    with tile.TileContext(nc) as tc:
        with tc.tile_pool(name="dram", bufs=1, space="DRAM") as dram:
            def din(name, shape, dt=F32):
                return dram.tile(shape, dt, kind="ExternalInput", name=name,
                                 uniquify=False)

            featq = din("featq", [BL, NCH, 128, KQ, CHUNK], F8)
            featres = din("featres", [BL, NCH, 128, KQ, CHUNK], BF16)
            tokT_l = din("tokT_l", [128, KKV, BL, NT], F8)
            tokTz_l = din("tokTz_l", [128, KKV, BL, 2, 128], F8)
            wq = din("wq", [128, KQ, DQ], F8)
            wk = din("wk", [128, KKV, DQ], F8)
            wvz = din("wvz", [128, KKV, 2, NPAIR // 2, 256], F8)
            wout = din("wout", [128, KQ, DQ], F8)
            blob_l = din("blob", [128, _NBLOB], U8)
            kz0_l = din("kz0", [128, NPAIR, 2, BL, 128], F8)
            out_l = dram.tile([BL, NCH, 128, KQ, CHUNK], BF16,
                              kind="ExternalOutput", name="out_l",
                              uniquify=False)

            with ExitStack() as body_ctx:
                global _body_ctx
                _body_ctx = body_ctx
                _body(nc, tc, featq, featres, tokT_l, tokTz_l, wq, wk, wvz,
                      wout, blob_l, kz0_l, out_l)
    nc.compile()
    return nc


def _body(nc, tc, featq, featres, tokT_l, tokTz_l, wq, wk, wvz, wout,
          blob_l, kz0_l, out_l):
    MULT = mybir.AluOpType.mult
    ADD = mybir.AluOpType.add
    EXP = mybir.ActivationFunctionType.Exp

    ctx = _body_ctx
    consts = ctx.enter_context(tc.tile_pool(name="consts", bufs=1))
    kside = ctx.enter_context(tc.tile_pool(name="kside", bufs=1))
    featp = ctx.enter_context(tc.tile_pool(name="featp", bufs=2))
    frp = ctx.enter_context(tc.tile_pool(name="frp", bufs=2))
    qp = ctx.enter_context(tc.tile_pool(name="qp", bufs=2))
    qsbp = ctx.enter_context(tc.tile_pool(name="qsbp", bufs=3))
    ep = ctx.enter_context(tc.tile_pool(name="ep", bufs=2))
    atp = ctx.enter_context(tc.tile_pool(name="atp", bufs=2))
    outp = ctx.enter_context(tc.tile_pool(name="outp", bufs=2))
    rp = ctx.enter_context(tc.tile_pool(name="rp", bufs=2))

    # single-bank tiles with a 4-deep rotation decouple the PE from the
    # DVE/ACT consumer latency
    pp2 = ctx.enter_context(tc.tile_pool(name="pp2", bufs=4, space="PSUM"))
    attn = ctx.enter_context(tc.tile_pool(name="attn", bufs=3, space="PSUM"))
    dp = ctx.enter_context(tc.tile_pool(name="dp", bufs=1, space="PSUM"))

    # ---- constant loads ------------------------------------------------
    # sync ring: the PE-critical path in need-order; gpsimd ring: blob +
    # kz zeros + per-chunk featres + output writes; scalar ring: compute
    # only (ktd/vbd copies, EXP, staging).
    tokT_sb = consts.tile([128, KKV, BL, NT], F8)
    nc.sync.dma_start(out=tokT_sb, in_=tokT_l[:])
    wk_sb = consts.tile([128, KKV, DQ], F8)
    nc.sync.dma_start(out=wk_sb, in_=wk[:])

    blob_sb = consts.tile([128, _NBLOB], U8)
    nc.gpsimd.dma_start(out=blob_sb, in_=blob_l[:])
    eps_sb = blob_sb[:, _O_EPS:_O_EPS + 4].bitcast(F32)
    neps_sb = blob_sb[:, _O_NEPS:_O_NEPS + 4].bitcast(F32)
    ck2_sb = blob_sb[:, _O_CK:_O_CK + 2 * NT].bitcast(BF16)
    sk2_sb = blob_sb[:, _O_SK:_O_SK + 2 * NT].bitcast(BF16)
    dlhs_sb = blob_sb[:, _O_DLHS:_O_DLHS + NPAIR * 16].bitcast(F8) \
        .rearrange("p (j m) -> p j m", j=NPAIR)
    ident_sb = blob_sb[:, _O_IDENT:_O_IDENT + 256].bitcast(BF16)
    blhs_sb = blob_sb[0:H, _O_BLHS:_O_BLHS + NPAIR * 256].bitcast(BF16) \
        .rearrange("p (j m) -> p j m", j=NPAIR)
    ctab_sb = blob_sb[:, _O_CTAB:_O_CTAB + 2 * NQ].bitcast(BF16)
    stab_sb = blob_sb[:, _O_STAB:_O_STAB + 2 * NQ].bitcast(BF16)

    wq_sb = consts.tile([128, KQ, DQ], F8)
    wout_sb = consts.tile([128, KQ, DQ], F8)
    tokTz_sb = consts.tile([128, KKV, BL, 2, 128], F8)
    wvz_sb = consts.tile([128, KKV, 2, NPAIR // 2, 256], F8)

    chunks = [(b, c) for b in range(BL) for c in range(NCH)]
    st = {}
    state = {}

    # ---- pipeline stages ------------------------------------------------
    def stage_qproj(i, mid_cb=None):
        b, c = chunks[i]
        p0 = c * CHUNK
        featb = featp.tile([128, KQ, CHUNK], F8, tag="featb", name=f"fb{i}")
        if i == 0:
            nc.sync.dma_start(out=wq_sb, in_=wq[:])
        nc.sync.dma_start(out=featb, in_=featq[b, c])
        qcs = qp.tile([128, NPAIR, 2, CHUNK], F8, tag="qcs", name=f"qcs{i}")
        for m in range(KQ):
            qps = pp2.tile([128, CHUNK], F32, tag="pp2", name=f"qp{i}_{m}")
            for t in range(KQ // 2):
                nc.tensor.matmul(qps,
                                 wq_sb[:, 2 * t:2 * t + 2,
                                       m * 128:(m + 1) * 128],
                                 featb[:, 2 * t:2 * t + 2, :],
                                 start=(t == 0), stop=(t == KQ // 2 - 1),
                                 perf_mode=DR)
            if m < 4:
                nc.vector.tensor_mul(qcs[:, m, 0, :], qps,
                                     ctab_sb[:, p0:p0 + CHUNK])
                nc.vector.tensor_mul(qcs[:, m, 1, :], qps,
                                     stab_sb[:, p0:p0 + CHUNK])
            else:
                qsb = qsbp.tile([128, CHUNK], BF16, tag="qsb",
                                name=f"qsb{i}_{m}")
                nc.scalar.copy(out=qsb, in_=qps)
                nc.vector.tensor_mul(qcs[:, m, 0, :], qsb,
                                     ctab_sb[:, p0:p0 + CHUNK])
                nc.vector.tensor_mul(qcs[:, m, 1, :], qsb,
                                     stab_sb[:, p0:p0 + CHUNK])
            if m == 2 and mid_cb is not None:
                mid_cb()
        st[i] = dict(qcs=qcs)

    def stage_qk(i):
        b, c = chunks[i]
        s = st[i]
        qcs = s["qcs"]
        kz_sb = state["kz"]
        # residual source for this chunk's out-projection (gpsimd ring);
        # deferred to here so it stays out of the head DMA burst
        fres = frp.tile([128, KQ, CHUNK], BF16, tag="fres", name=f"fr{i}")
        nc.gpsimd.dma_start(out=fres, in_=featres[b, c])
        s["fres"] = fres
        e_sb = ep.tile([128, NPAIR, CHUNK], F8, tag="e", name=f"e{i}")
        dps = dp.tile([16, CHUNK], F32, tag="den", name=f"d{i}")

        def qk1(j):
            # one zero-padded block-diagonal DR matmul per head pair:
            # k-tile 0 = blockdiag(kA_2j, kA_2j+1), k-tile 1 = kB pair;
            # rhs k-tiles are the natural [q*cos], [q*sin] pair blocks
            sps = attn.tile([128, CHUNK], F32, tag="attn", name=f"s{i}_{j}")
            nc.tensor.matmul(sps, kz_sb[:, j, :, b, :], qcs[:, j, :, :],
                             start=True, stop=True, perf_mode=DR)
            nc.scalar.activation(out=e_sb[:, j, :], in_=sps, func=EXP)

        def denom(t):
            nc.tensor.matmul(dps, dlhs_sb[:, 2 * t:2 * t + 2, :],
                             e_sb[:, 2 * t:2 * t + 2, :],
                             start=(t == 0), stop=(t == NPAIR // 2 - 1),
                             perf_mode=DR)

        for j in range(NPAIR):
            qk1(j)
            if j % 2 == 1:
                denom((j - 1) // 2)
        s["e"], s["dps"] = e_sb, dps

    def stage_recip(i):
        s = st[i]
        r32 = rp.tile([16, CHUNK], F32, tag="r32", name=f"r32_{i}")
        nc.vector.reciprocal_approx_fast(out=r32, in_=s["dps"])
        r_sb = rp.tile([16, CHUNK], BF16, tag="r", name=f"r{i}")
        nc.scalar.copy(out=r_sb, in_=r32)
        s["r"] = r_sb

    def stage_avbc(i):
        b, c = chunks[i]
        s = st[i]
        e_sb, r_sb = s["e"], s["r"]
        vbd_sb = state["vbd"]
        attnT = atp.tile([128, NPAIR, CHUNK], F8, tag="attnT", name=f"at{i}")

        def av(j):
            aps = attn.tile([128, CHUNK], F32, tag="attn", name=f"a{i}_{j}")
            nc.tensor.matmul(aps, vbd_sb[:, b, j, :], e_sb[:, j, :],
                             start=True, stop=True)
            return aps

        def bcast(j):
            bps = attn.tile([128, CHUNK], F32, tag="attn", name=f"b{i}_{j}")
            nc.tensor.matmul(bps, blhs_sb[:, j, :], r_sb[0:H, :],
                             start=True, stop=True)
            # stage to SBUF (DVE may read only one PSUM operand)
            bcs = rp.tile([128, CHUNK], F32, tag="bcs", bufs=3,
                          name=f"bc{i}_{j}")
            if j < 4:
                nc.scalar.copy(out=bcs, in_=bps)
            else:
                nc.vector.tensor_copy(bcs, bps)
            return bcs

        av_t, bc_t = {}, {}
        for j in range(NPAIR):
            av_t[j] = av(j)
            bc_t[j] = bcast(j)
            if j >= 1:
                nc.vector.tensor_mul(attnT[:, j - 1, :],
                                     av_t[j - 1], bc_t[j - 1])
        nc.vector.tensor_mul(attnT[:, NPAIR - 1, :],
                             av_t[NPAIR - 1], bc_t[NPAIR - 1])
        s["attnT"] = attnT

    def stage_oproj(i):
        b, c = chunks[i]
        s = st[i]
        attnT = s["attnT"]
        fres = s["fres"]
        o2 = outp.tile([128, KQ, CHUNK], BF16, tag="o2", name=f"o{i}")
        for m in range(KQ):
            ops = pp2.tile([128, CHUNK], F32, tag="pp2", name=f"op{i}_{m}")
            for t in range(KQ // 2):
                nc.tensor.matmul(ops,
                                 wout_sb[:, 2 * t:2 * t + 2,
                                         m * 128:(m + 1) * 128],
                                 attnT[:, 2 * t:2 * t + 2, :],
                                 start=(t == 0), stop=False,
                                 perf_mode=DR)
            # residual + bias ride the PE: accumulate featres via identity
            nc.tensor.matmul(ops, ident_sb, fres[:, m, :],
                             start=False, stop=True)
            nc.scalar.copy(out=o2[:, m, :], in_=ops)
        # one contiguous 786KB write per chunk; sync ring (HWDGE) so the
        # gpsimd SWDGE drain isn't the kernel's closing act
        nc.sync.dma_start(out=out_l[b, c], in_=o2)

    # ---- emission -------------------------------------------------------
    def scoped(fn, tag, i):
        with nc.named_scope(f"{tag}{i}"):
            fn(i)

    _ph0 = nc.named_scope("ph0")
    _ph0.__enter__()
    # kz zero prefill from DRAM (cheaper than a 2.7us gpsimd memset)
    kz_sb = kside.tile([128, NPAIR, 2, BL, 128], F8, tag="kz")
    nc.gpsimd.dma_start(out=kz_sb, in_=kz0_l[:])
    state["kz"] = kz_sb
    # kT as fp8 DR pair + single per head-pair block (12 instructions)
    ktd_sb = kside.tile([128, NPAIR, BL, NT], BF16)
    for hp in range(NPAIR):
        kp = pp2.tile([128, 2, 128], F32, tag="pp2", name=f"kp{hp}")
        nc.tensor.matmul(kp[:, 0, :],
                         wk_sb[:, 0:2, hp * 128:(hp + 1) * 128],
                         tokT_sb[:, 0:2],
                         start=True, stop=False, perf_mode=DR)
        nc.tensor.matmul(kp[:, 0, :],
                         wk_sb[:, 2, hp * 128:(hp + 1) * 128],
                         tokT_sb[:, 2], start=False, stop=True)
        nc.scalar.copy(out=ktd_sb[:, hp], in_=kp[:, 0, :])
    _ph0.__exit__(None, None, None)

    # k RoPE t1/t2 on DVE (in front of chunk-0's q-RoPE in the queue); the
    # k tables are read through 0-stride broadcast APs (pair, batch axes)
    _ph1 = nc.named_scope("ph1")
    _ph1.__enter__()
    ck_v = ck2_sb.unsqueeze(1).unsqueeze(2).broadcast_to([128, NPAIR, BL, NT])
    sk_v = sk2_sb.unsqueeze(1).unsqueeze(2).broadcast_to([128, NPAIR, BL, NT])
    t12 = kside.tile([128, 2, NPAIR, BL, NT], BF16, tag="t12")
    nc.vector.tensor_mul(t12[:, 0], ktd_sb, ck_v)
    nc.vector.tensor_mul(t12[:, 1], ktd_sb, sk_v)
    t12s = kside.tile([128, 2, NPAIR, BL, NT], BF16, tag="t12s")
    _ph1.__exit__(None, None, None)

    def kz_scatter(b):
        # kA = t1 + eps*sigma(t2) into k-tile 0; kB = t2 - eps*sigma(t1)
        # into k-tile 1 (TensorScalarPtr is DVE-only)
        for half in range(2):
            pm = slice(64 * half, 64 * half + 64)
            nc.vector.scalar_tensor_tensor(
                out=kz_sb[pm, :, 0, b, pm], in0=t12s[pm, 1, :, b, :],
                scalar=eps_sb[pm], in1=t12[pm, 0, :, b, :],
                op0=MULT, op1=ADD)
            nc.vector.scalar_tensor_tensor(
                out=kz_sb[pm, :, 1, b, pm], in0=t12s[pm, 0, :, b, :],
                scalar=neps_sb[pm], in1=t12[pm, 1, :, b, :],
                op0=MULT, op1=ADD)

    def ph1_mid():
        # sigma DMAs (gpsimd ring) + batch-0 scatter, emitted mid-qproj so
        # DVE finishes kz right when the qk0 matmuls need it
        with nc.named_scope("ph1b"):
            _sigma_dma(nc, t12s, t12, eng=nc.gpsimd)
            kz_scatter(0)

    scoped(lambda i: stage_qproj(i, mid_cb=ph1_mid), "qp", 0)

    # V-side after qp0 so its sync-ring loads trail wq/featb chunk 0
    _ph2 = nc.named_scope("ph2")
    _ph2.__enter__()
    nc.sync.dma_start(out=tokTz_sb, in_=tokTz_l[:])
    nc.sync.dma_start(out=wvz_sb, in_=wvz[:])
    nc.sync.dma_start(out=wout_sb, in_=wout[:])
    # V: zero-padded block-diagonal DR matmuls (tokens duplicated across the
    # two k-tiles host-side), two head pairs per matmul (N=256) ->
    # vbd = [v_even | 0; 0 | v_odd] per pair.
    vbd_sb = kside.tile([128, BL, NPAIR, 128], F8, tag="vbd")
    state["vbd"] = vbd_sb
    for b in range(BL):
        for g in range(NPAIR // 2):
            vbp = pp2.tile([128, 2, 128], F32, tag="pp2", name=f"vb{b}_{g}")
            for kc in range(KKV):
                nc.tensor.matmul(vbp[:], tokTz_sb[:, kc, b, :, :],
                                 wvz_sb[:, kc, :, g, :],
                                 start=(kc == 0), stop=(kc == KKV - 1),
                                 perf_mode=DR)
            nc.scalar.copy(out=vbd_sb[:, b, 2 * g:2 * g + 2, :], in_=vbp[:])
    _ph2.__exit__(None, None, None)

    scoped(stage_qk, "qk", 0)

    n = len(chunks)
    for i in range(n):
        scoped(stage_recip, "rc", i)
        scoped(stage_avbc, "av", i)
        if i + 1 < n:
            scoped(stage_qproj, "qp", i + 1)
            if i == 0:
                with nc.named_scope("kz1"):
                    kz_scatter(1)
            scoped(stage_qk, "qk", i + 1)
        scoped(stage_oproj, "op", i)


_NC_CACHE = {}


def _get_nc():
    if "nc" not in _NC_CACHE:
        _NC_CACHE["nc"] = build(debug=False)
    return _NC_CACHE["nc"]


def _prep_in_maps(feat, tokens, Wq, Wkv, Wout, bout):
    feat = np.ascontiguousarray(feat, dtype=np.float32).reshape(B, DQ, NQ)
    tokens = np.ascontiguousarray(tokens, dtype=np.float32)
    wv = np.asarray(Wkv)[:, DQ:]                       # [DKV, DQ]
    wvz = np.zeros((KKV, 128, NPAIR, 2, 128), np.float32)
    wvr = wv.reshape(KKV, 128, H, HD)
    for j in range(NPAIR):
        wvz[:, :, j, 0, :HD] = wvr[:, :, 2 * j, :]
        wvz[:, :, j, 1, HD:] = wvr[:, :, 2 * j + 1, :]
    # [kc, p, j, t, 128] -> [p, kc, t, g, 256] with two head pairs per group
    wvz2 = wvz.transpose(1, 0, 3, 2, 4).reshape(128, KKV, 2, NPAIR // 2, 256)
    shared = dict(
        wq=np.ascontiguousarray(
            np.asarray(Wq).reshape(KQ, 128, DQ).transpose(1, 0, 2))
            .astype(NPF8),
        wk=np.ascontiguousarray(
            np.asarray(Wkv)[:, :DQ].reshape(KKV, 128, DQ).transpose(1, 0, 2))
            .astype(NPF8),
        wvz=np.ascontiguousarray(wvz2).astype(NPF8),
        wout=np.ascontiguousarray(
            np.asarray(Wout).reshape(KQ, 128, DQ).transpose(1, 0, 2))
            .astype(NPF8),
        blob=_const_blob(),
        kz0=np.zeros((128, NPAIR, 2, BL, 128), NPF8),
    )
    bias = np.asarray(bout, dtype=np.float32).reshape(KQ, 128)  # [kc, p]
    in_maps = []
    for cid in range(NCORES):
        sl = slice(BL * cid, BL * (cid + 1))
        fl = np.ascontiguousarray(
            feat[sl].reshape(BL, KQ, 128, NQ).transpose(0, 2, 1, 3))
        fo = fl + bias.T[None, :, :, None]
        # per-chunk contiguous layout [BL, NCH, 128, KQ, CHUNK]
        fq = fl.reshape(BL, 128, KQ, NCH, CHUNK).transpose(0, 3, 1, 2, 4)
        fo = fo.reshape(BL, 128, KQ, NCH, CHUNK).transpose(0, 3, 1, 2, 4)
        tk = tokens[sl].reshape(BL, NT, KKV, 128).transpose(3, 2, 0, 1)
        tkz = np.zeros((128, KKV, BL, 2, 128), np.float32)
        tkz[:, :, :, 0, :NT] = tk
        tkz[:, :, :, 1, NT:] = tk
        in_maps.append(dict(
            featq=np.ascontiguousarray(fq).astype(NPF8),
            featres=np.ascontiguousarray(fo).astype(NPBF),
            tokT_l=np.ascontiguousarray(tk).astype(NPF8),
            tokTz_l=np.ascontiguousarray(tkz).astype(NPF8),
            **shared))
    return in_maps


def _install_ntff_hook():
    """The container's antenv lacks axon_hooks; register the NTFF profile
    hook from trn_agent_boot ourselves so trace=True yields HW exec times."""
    import types

    import antenv
    from trn_agent_boot.trn_boot import _ntff_profile_via_ctypes

    mod = types.ModuleType("antenv.axon_hooks")
    state = {"hook": None}
    mod.set_axon_ntff_profile_hook = lambda h: state.__setitem__("hook", h)
    mod.get_axon_ntff_profile_hook = lambda: state["hook"]
    sys.modules["antenv.axon_hooks"] = mod
    antenv.axon_hooks = mod
    mod.set_axon_ntff_profile_hook(
        _ntff_profile_via_ctypes("/opt/axon/libaxon_pjrt.so"))
    # the S3 artifact upload has no credentials here; make it a no-op
    import concourse.bass_utils as bu
    bu.upload_artifacts = lambda tmpdir: f"local:{tmpdir}"


def run(inputs, trace=False, trace_cores=None):
    nc = _get_nc()
    if trace:
        try:
            _install_ntff_hook()
        except Exception as e:  # profiling is best-effort
            print(f"ntff hook install failed: {e}", file=sys.stderr)
            trace = False
    in_maps = _prep_in_maps(**inputs)
    res = run_bass_kernel_spmd(nc, in_maps, core_ids=list(range(NCORES)),
                               trace=trace, trace_cores=trace_cores)
    outs = []
    for r in res.results:
        ol = np.asarray(r["out_l"]).astype(np.float32)
        # [BL, NCH, 128, KQ, CHUNK] -> [BL, DQ, NQ]
        ol = ol.transpose(0, 3, 2, 1, 4).reshape(BL, DQ, NQ)
        outs.append(ol.reshape(BL, DQ, T, HP, WP))
    return np.ascontiguousarray(np.concatenate(outs, axis=0)), res


def kernel(**inputs):
    return run(inputs, trace=False)[0]
